# revision 4
# baseline (speedup 1.0000x reference)
"""Trainium2 Bass kernel for nn_ApproximationLayer_84327387890499.

Op: zero bit 62 (exponent MSB) of the IEEE-754 double bit pattern of
x[b, r, c] for (r, c) in rows x cols; passthrough elsewhere.

Everything is done on the int32 view of the f64 tensor [B, R, 2C]:
clearing bit 62 of a double == AND-ing its high int32 word with
0xBFFFFFFF. Sharding is data parallel over batch: 256 -> 32 per core
on 8 cores.

Fast path (rows/cols are contiguous ranges, the harness case): the
region outside the masked block is moved with direct DRAM->DRAM DMA
copies (no SBUF, no compute) split across both HWDGE rings; only the
tiny masked block (256 KiB/core for 32x32) goes through SBUF, gets
AND-ed with a parity mask on VectorE, and is stored back via the SWDGE
ring. All output regions are disjoint, so nothing serializes.

Fallback (arbitrary rows/cols): stream the whole shard through SBUF in
[128, 4096] tiles and AND with a host-built full mask.
"""
import numpy as np

import concourse.bass as bass
import concourse.tile as tile
from concourse import bacc, mybir
from concourse.bass_utils import run_bass_kernel_spmd

B, R, C = 256, 512, 512
C2 = 2 * C                        # int32 columns per row
N_CORES = 8
B_SHARD = B // N_CORES            # 32 batches per core

HI_AND = np.int32(-1073741825)    # 0xBFFFFFFF: clears bit 30 of the high word

_programs = {}


def _build_general():
    """Uniform pass: out = x & M for a full [R, C2] mask M."""
    if "gen" in _programs:
        return _programs["gen"]
    F = 4096
    nc = bacc.Bacc("TRN2", target_bir_lowering=False, debug=False)
    x_ext = nc.declare_dram_parameter("x", [B_SHARD, 128, F], mybir.dt.int32, isOutput=False)
    m_ext = nc.declare_dram_parameter("m", [128, F], mybir.dt.int32, isOutput=False)
    out_ext = nc.declare_dram_parameter("out", [B_SHARD, 128, F], mybir.dt.int32, isOutput=True)
    x_ap, m_ap, out_ap = x_ext.ap(), m_ext.ap(), out_ext.ap()

    with tile.TileContext(nc) as tc:
        with tc.tile_pool(name="mask", bufs=1) as mask_pool, \
             tc.tile_pool(name="x", bufs=6) as x_pool:
            mt = mask_pool.tile([128, F], mybir.dt.int32)
            nc.sync.dma_start(out=mt[:], in_=m_ap[:])
            for n in range(B_SHARD):
                xt = x_pool.tile([128, F], mybir.dt.int32)
                nc.sync.dma_start(out=xt[:], in_=x_ap[n])
                nc.vector.tensor_tensor(
                    out=xt[:], in0=xt[:], in1=mt[:],
                    op=mybir.AluOpType.bitwise_and,
                )
                nc.scalar.dma_start(out=out_ap[n], in_=xt[:])
    nc.compile()
    _programs["gen"] = nc
    return nc


def _build_block(r0, nr, c0, ncc):
    """Contiguous-block fast path for rows r0:r0+nr, cols c0:c0+ncc."""
    key = ("blk", r0, nr, c0, ncc)
    if key in _programs:
        return _programs[key]
    r1 = r0 + nr
    f0, f1 = 2 * c0, 2 * (c0 + ncc)           # int32 col range of the block
    n_elem = B_SHARD * nr * (f1 - f0)          # block int32 elements per core
    assert n_elem % 128 == 0
    FX = n_elem // 128                         # SBUF free dim for the block

    nc = bacc.Bacc("TRN2", target_bir_lowering=False, debug=False)
    x_ext = nc.declare_dram_parameter("x", [B_SHARD, R, C2], mybir.dt.int32, isOutput=False)
    m_ext = nc.declare_dram_parameter("m", [128, FX], mybir.dt.int32, isOutput=False)
    out_ext = nc.declare_dram_parameter("out", [B_SHARD, R, C2], mybir.dt.int32, isOutput=True)
    x_ap, out_ap = x_ext.ap(), out_ext.ap()

    fix_sb = nc.alloc_sbuf_tensor("fix_sb", [128, FX], mybir.dt.int32)
    msk_sb = nc.alloc_sbuf_tensor("msk_sb", [128, FX], mybir.dt.int32)

    # Bulk DRAM->DRAM pieces. The masked-row band r0:r1 is copied whole
    # (full columns -> big contiguous chunks); the fixup overwrites the
    # block inside it afterwards. Rows outside the band are split by
    # batch ranges and balanced across the two HWDGE rings by bytes.
    # Each piece: (dst, src, bytes).
    band_bytes = B_SHARD * nr * C2 * 4
    band = (out_ap[:, r0:r1, :], x_ap[:, r0:r1, :], band_bytes)

    bulk = []
    hb = B_SHARD // 2
    for blo, bhi in ((0, hb), (hb, B_SHARD)):
        nbatch = bhi - blo
        if r0 > 0:
            bb = nbatch * r0 * C2 * 4
            bulk.append((out_ap[blo:bhi, 0:r0, :], x_ap[blo:bhi, 0:r0, :], bb))
        if r1 < R:
            bb = nbatch * (R - r1) * C2 * 4
            bulk.append((out_ap[blo:bhi, r1:R, :], x_ap[blo:bhi, r1:R, :], bb))
    # Greedy byte-balance: scalar ring starts with the band copy.
    sync_pieces, scalar_pieces = [], []
    loads = [0, band_bytes]
    for dst, src, bb in sorted(bulk, key=lambda p: -p[2]):
        if loads[0] <= loads[1]:
            sync_pieces.append((dst, src))
            loads[0] += bb
        else:
            scalar_pieces.append((dst, src))
            loads[1] += bb

    with (
        nc.Block() as block,
        nc.semaphore("s_ld") as s_ld,
        nc.semaphore("s_cpa") as s_cpa,
        nc.semaphore("s_cpb") as s_cpb,
        nc.semaphore("s_band") as s_band,
        nc.semaphore("s_fx") as s_fx,
        nc.semaphore("s_v") as s_v,
    ):
        @block.sync
        def _(sync: bass.BassEngine):
            for dst, src in sync_pieces:
                sync.dma_start(out=dst, in_=src).then_inc(s_cpa, 16)
            sync.wait_ge(s_cpa, 16 * len(sync_pieces))

        @block.scalar
        def _(scalar: bass.BassEngine):
            scalar.dma_start(out=band[0], in_=band[1]).then_inc(s_band, 16)
            for dst, src in scalar_pieces:
                scalar.dma_start(out=dst, in_=src).then_inc(s_cpb, 16)
            scalar.wait_ge(s_cpb, 16 * len(scalar_pieces))

        @block.vector
        def _(vector: bass.BassEngine):
            vector.wait_ge(s_ld, 32)
            vector.tensor_tensor(
                out=fix_sb.ap()[:], in0=fix_sb.ap()[:], in1=msk_sb.ap()[:],
                op=mybir.AluOpType.bitwise_and,
            ).then_inc(s_v, 1)

        @block.gpsimd
        def _(gpsimd: bass.BassEngine):
            gpsimd.dma_start(out=fix_sb.ap()[:], in_=x_ap[:, r0:r1, f0:f1]).then_inc(s_ld, 16)
            gpsimd.dma_start(out=msk_sb.ap()[:], in_=m_ext.ap()[:]).then_inc(s_ld, 16)
            gpsimd.wait_ge(s_v, 1)
            gpsimd.wait_ge(s_band, 16)
            gpsimd.dma_start(out=out_ap[:, r0:r1, f0:f1], in_=fix_sb.ap()[:]).then_inc(s_fx, 16)
            gpsimd.wait_ge(s_fx, 16)

    nc.compile()
    _programs[key] = nc
    return nc


def _contiguous_start(idx):
    """Return start if idx == arange(start, start+len), else None."""
    if idx.size == 0:
        return None
    start = int(idx[0])
    if np.array_equal(idx, np.arange(start, start + idx.size)):
        return start
    return None


def kernel(x, rows, cols):
    x = np.ascontiguousarray(np.asarray(x))
    rows = np.asarray(rows).astype(np.int64)
    cols = np.asarray(cols).astype(np.int64)
    assert x.shape == (B, R, C) and x.dtype == np.float64

    x_i32 = x.view(np.int32).reshape(B, R, C2)
    shards = x_i32.reshape(N_CORES, B_SHARD, R, C2)

    r0 = _contiguous_start(rows)
    c0 = _contiguous_start(cols)

    n_blk = B_SHARD * rows.size * 2 * cols.size
    if (
        r0 is not None and c0 is not None
        and n_blk % 128 == 0
        and (n_blk // 128) * 4 <= 64 * 1024  # fix + mask tiles must fit SBUF
    ):
        nr, ncc = rows.size, cols.size
        nc = _build_block(r0, nr, c0, ncc)
        n_elem = n_blk
        FX = n_elem // 128
        # Stream-order parity mask: within the block every (r, c) is
        # masked, and int32 stream position f has c-parity f%2
        # (high word at odd f) because 2*ncc is even.
        m_fix = np.full((128, FX), -1, dtype=np.int32)
        m_fix[:, 1::2] = HI_AND
        in_maps = [{"x": shards[i], "m": m_fix} for i in range(N_CORES)]
        res = run_bass_kernel_spmd(nc, in_maps, core_ids=list(range(N_CORES)))
        out = np.empty((N_CORES, B_SHARD, R, C2), dtype=np.int32)
        for i in range(N_CORES):
            out[i] = res.results[i]["out"]
        return out.reshape(B, R, C2).view(np.float64).reshape(B, R, C)

    # General fallback: full-tensor AND with a host-built mask.
    F = 4096
    m = np.full((R, C2), -1, dtype=np.int32)
    m[np.ix_(rows, 2 * cols + 1)] = HI_AND
    m_tiled = m.reshape(128, F)
    nc = _build_general()
    xs = x_i32.reshape(N_CORES, B_SHARD, 128, F)
    in_maps = [{"x": xs[i], "m": m_tiled} for i in range(N_CORES)]
    res = run_bass_kernel_spmd(nc, in_maps, core_ids=list(range(N_CORES)))
    out = np.empty((N_CORES, B_SHARD, 128, F), dtype=np.int32)
    for i in range(N_CORES):
        out[i] = res.results[i]["out"]
    return out.reshape(B, R, C2).view(np.float64).reshape(B, R, C)


# revision 5
# speedup vs baseline: 1.5012x; 1.5012x over previous
"""Trainium2 Bass kernel for nn_ApproximationLayer_84327387890499.

Op: zero bit 62 (exponent MSB) of the IEEE-754 double bit pattern of
x[b, r, c] for (r, c) in rows x cols; passthrough elsewhere.

Everything runs on the int32 view of the f64 tensor [B, R, 2C]:
clearing bit 62 of a double == AND-ing its high int32 word with
0xBFFFFFFF. Sharding is data parallel over batch: 256 -> 32 per core
on 8 cores.

Fast path (rows/cols form contiguous ranges - the harness case):
1. One whole-shard DRAM->DRAM copy. Contiguous src/dst collapse to a
   2D access pattern with 64 KiB descriptors - measured ~215 us for
   64 MiB on one HWDGE ring (~620 GB/s HBM read+write per core, the
   HBM cap; 3D strided APs run 1.5-2x slower, so they are avoided
   entirely).
2. In parallel, the tiny masked block (256 KiB) is loaded to SBUF, its
   odd (high-word) int32 lanes are AND-ed with an immediate on
   VectorE, and the result overwrites the block region after the bulk
   copy lands (semaphore-ordered WAW, ~small tail).

Fallback (anything else): stream the whole shard through SBUF in
[128, 4096] tiles AND-ed against a host-built full mask.
"""
import numpy as np

import concourse.bass as bass
import concourse.tile as tile
from concourse import bacc, mybir
from concourse.bass_utils import run_bass_kernel_spmd

B, R, C = 256, 512, 512
C2 = 2 * C                        # int32 columns per row
N_CORES = 8
B_SHARD = B // N_CORES            # 32 batches per core

HI_AND = np.int32(-1073741825)    # 0xBFFFFFFF: clears bit 30 of the high word

_programs = {}


def _build_general():
    """Uniform pass: out = x & M for a full [R, C2] mask M."""
    if "gen" in _programs:
        return _programs["gen"]
    F = 4096
    nc = bacc.Bacc("TRN2", target_bir_lowering=False, debug=False)
    x_ext = nc.declare_dram_parameter("x", [B_SHARD, 128, F], mybir.dt.int32, isOutput=False)
    m_ext = nc.declare_dram_parameter("m", [128, F], mybir.dt.int32, isOutput=False)
    out_ext = nc.declare_dram_parameter("out", [B_SHARD, 128, F], mybir.dt.int32, isOutput=True)
    x_ap, m_ap, out_ap = x_ext.ap(), m_ext.ap(), out_ext.ap()

    with tile.TileContext(nc) as tc:
        with tc.tile_pool(name="mask", bufs=1) as mask_pool, \
             tc.tile_pool(name="x", bufs=6) as x_pool:
            mt = mask_pool.tile([128, F], mybir.dt.int32)
            nc.sync.dma_start(out=mt[:], in_=m_ap[:])
            for n in range(B_SHARD):
                xt = x_pool.tile([128, F], mybir.dt.int32)
                nc.sync.dma_start(out=xt[:], in_=x_ap[n])
                nc.vector.tensor_tensor(
                    out=xt[:], in0=xt[:], in1=mt[:],
                    op=mybir.AluOpType.bitwise_and,
                )
                nc.scalar.dma_start(out=out_ap[n], in_=xt[:])
    nc.compile()
    _programs["gen"] = nc
    return nc


def _build_block(r0, nr, c0, ncc):
    """Contiguous-block fast path for rows r0:r0+nr, cols c0:c0+ncc."""
    key = ("blk", r0, nr, c0, ncc)
    if key in _programs:
        return _programs[key]
    r1 = r0 + nr
    f0, f1 = 2 * c0, 2 * (c0 + ncc)            # int32 col range of the block
    n_elem = B_SHARD * nr * (f1 - f0)           # block int32 elements per core
    assert n_elem % 128 == 0
    FX = n_elem // 128                          # SBUF free dim for the block

    nc = bacc.Bacc("TRN2", target_bir_lowering=False, debug=False)
    x_ext = nc.declare_dram_parameter("x", [B_SHARD, R, C2], mybir.dt.int32, isOutput=False)
    out_ext = nc.declare_dram_parameter("out", [B_SHARD, R, C2], mybir.dt.int32, isOutput=True)
    x_ap, out_ap = x_ext.ap(), out_ext.ap()
    fix = nc.alloc_sbuf_tensor("fix", [128, FX], mybir.dt.int32)

    with (
        nc.Block() as block,
        nc.semaphore("s_ld") as s_ld,
        nc.semaphore("s_cp") as s_cp,
        nc.semaphore("s_st") as s_st,
        nc.semaphore("s_v") as s_v,
    ):
        @block.sync
        def _(sync: bass.BassEngine):
            sync.dma_start(out=out_ap[:], in_=x_ap[:]).then_inc(s_cp, 16)
            sync.wait_ge(s_cp, 16)

        @block.scalar
        def _(scalar: bass.BassEngine):
            scalar.dma_start(out=fix.ap()[:], in_=x_ap[:, r0:r1, f0:f1]).then_inc(s_ld, 16)
            scalar.wait_ge(s_v, 1)
            scalar.wait_ge(s_cp, 16)
            scalar.dma_start(out=out_ap[:, r0:r1, f0:f1], in_=fix.ap()[:]).then_inc(s_st, 16)
            scalar.wait_ge(s_st, 16)

        @block.vector
        def _(vector: bass.BassEngine):
            vector.wait_ge(s_ld, 16)
            # Odd int32 stream positions are the high words (f1-f0 is even).
            vector.tensor_single_scalar(
                out=fix.ap()[:, 1::2], in_=fix.ap()[:, 1::2],
                scalar=int(HI_AND), op=mybir.AluOpType.bitwise_and,
            ).then_inc(s_v, 1)

    nc.compile()
    _programs[key] = nc
    return nc


def _contiguous_start(idx):
    """Return start if set(idx) == {start .. start+n-1}, else None."""
    u = np.unique(idx)
    if u.size == 0:
        return None
    start = int(u[0])
    if np.array_equal(u, np.arange(start, start + u.size)):
        return start, u.size
    return None


def kernel(x, rows, cols):
    x = np.ascontiguousarray(np.asarray(x))
    rows = np.asarray(rows).astype(np.int64)
    cols = np.asarray(cols).astype(np.int64)
    assert x.shape == (B, R, C) and x.dtype == np.float64

    x_i32 = x.view(np.int32).reshape(B, R, C2)
    shards = x_i32.reshape(N_CORES, B_SHARD, R, C2)

    rc = _contiguous_start(rows)
    cc = _contiguous_start(cols)

    if rc is not None and cc is not None:
        r0, nr = rc
        c0, ncc = cc
        n_elem = B_SHARD * nr * 2 * ncc
        if n_elem % 128 == 0 and (n_elem // 128) * 4 <= 128 * 1024:
            nc = _build_block(r0, nr, c0, ncc)
            in_maps = [{"x": shards[i]} for i in range(N_CORES)]
            res = run_bass_kernel_spmd(nc, in_maps, core_ids=list(range(N_CORES)))
            out = np.empty((N_CORES, B_SHARD, R, C2), dtype=np.int32)
            for i in range(N_CORES):
                out[i] = res.results[i]["out"]
            return out.reshape(B, R, C2).view(np.float64).reshape(B, R, C)

    # General fallback: full-tensor AND with a host-built mask.
    F = 4096
    m = np.full((R, C2), -1, dtype=np.int32)
    m[np.ix_(rows, 2 * cols + 1)] = HI_AND
    m_tiled = m.reshape(128, F)
    nc = _build_general()
    xs = x_i32.reshape(N_CORES, B_SHARD, 128, F)
    in_maps = [{"x": xs[i], "m": m_tiled} for i in range(N_CORES)]
    res = run_bass_kernel_spmd(nc, in_maps, core_ids=list(range(N_CORES)))
    out = np.empty((N_CORES, B_SHARD, 128, F), dtype=np.int32)
    for i in range(N_CORES):
        out[i] = res.results[i]["out"]
    return out.reshape(B, R, C2).view(np.float64).reshape(B, R, C)
